# revision 3
# baseline (speedup 1.0000x reference)
"""Trainium2 Bass kernel for nn_ModelMultitaskBinary (MMoE multitask loss).

Strategy: data-parallel over batch B=512 across 8 cores (64 samples/core),
params replicated, no collectives (host averages the 8x64 per-sample
losses). Zero-bias inputs take an fp8e4 DoubleRow program (all large
matmuls at 4x bf16 throughput in K=256 pairs); the per-(task,expert)
gated combination runs on the PE as yT[t] += eo_pair^T @ diag(gates),
fusing the transpose and the gating multiply. Power-of-two scale
management keeps fp8 activations near unit scale with exact rescales
folded into psum evacuations. Non-zero-bias inputs fall back to the
bf16 program.
"""
import os
import sys

for _p in ("/opt/trn_rl_repo", "/root/.axon_site/_ro/trn_rl_repo"):
    if os.path.isdir(_p) and _p not in sys.path:
        sys.path.insert(0, _p)

import numpy as np
import ml_dtypes
from contextlib import ExitStack
from itertools import cycle

import concourse.tile as tile
from concourse import mybir
from concourse.masks import make_identity

F32 = mybir.dt.float32
FP8 = mybir.dt.float8e4
E4 = ml_dtypes.float8_e4m3
AF = mybir.ActivationFunctionType
OP = mybir.AluOpType
AX = mybir.AxisListType
PM = mybir.MatmulPerfMode

NCORES = 8
B, C, T, H, E, EH, TH = 512, 30, 3, 512, 6, 512, 512
BSH = B // NCORES
R = BSH * C                # 1920 rows per core
NRT = R // 128             # 15 row tiles
KC = H // 128              # 4 k-chunks
TE = T * E                 # 18
RS = [(0, 512), (512, 1024), (1024, 1536), (1536, R)]
RG_RT = [(0, 4), (4, 8), (8, 12), (12, 15)]
LOSS_COEF = 0.01
WS = 2.0 ** 7              # weight pre-scale (host)

# power-of-two evacuation scales (exact in fp)
SC_H1 = 2.0 ** -5          # E_h1 = 4
SC_H = 2.0 ** -7           # E_h  = 4
SC_GLOG = 2.0 ** -9        # glog logical in f32
SC_EHS = 2.0 ** -6         # E_ehs = 8
SC_EO = 2.0 ** -5          # E_eo = 32
SC_YT = 1.0                # E_y  = 32
SC_TH = 2.0 ** -6          # E_th = 64
SC_LOG = 2.0 ** -13        # logits logical in f32

# engine rotation per evacuation stage: a=ACT, d=DVE, p=Pool
KNOBS_FP8 = {
    "h": "da",
    "ehs": "ada",
    "ehs2": "ada",
    "eo": "ada",
    "eo2": "ada",
    "yT": "d",
    "thT": "da",
    "diag": "pppp",
    "gat": "d",
}


def build_nc_fp8(knobs=None):
    import concourse.bass as bass  # noqa: F401
    from concourse import bacc

    kn = dict(KNOBS_FP8)
    if knobs:
        kn.update(knobs)
    rot = {k: cycle(v) for k, v in kn.items()}

    nc = bacc.Bacc(None, target_bir_lowering=False, debug=False)

    xT_d = nc.dram_tensor("xT", [128, KC * R], FP8, kind="ExternalInput")
    fc1w_d = nc.dram_tensor("fc1w", [128, KC * H], FP8, kind="ExternalInput")
    fc2w_d = nc.dram_tensor("fc2w", [128, KC * H], FP8, kind="ExternalInput")
    wg_d = nc.dram_tensor("wg", [128, KC * TE], FP8, kind="ExternalInput")
    ew1_d = nc.dram_tensor("ew1", [128, E * KC * EH], FP8, kind="ExternalInput")
    ew2_d = nc.dram_tensor("ew2", [128, E * KC * H], FP8, kind="ExternalInput")
    tw1_d = nc.dram_tensor("tw1", [128, T * KC * TH], FP8, kind="ExternalInput")
    tw2_d = nc.dram_tensor("tw2", [128, KC * T], FP8, kind="ExternalInput")
    scores_d = nc.dram_tensor("scores", [BSH, T * C], F32, kind="ExternalInput")
    srm_d = nc.dram_tensor("srm", [128, NRT * T], F32, kind="ExternalInput")
    sel_d = nc.dram_tensor("sel", [128, NRT * BSH], F32, kind="ExternalInput")
    selt_d = nc.dram_tensor("selt", [BSH, NRT * 128], F32, kind="ExternalInput")
    loss_d = nc.dram_tensor("loss", [BSH, 1], F32, kind="ExternalOutput")

    def ev(stage, dst, src, scale, relu=False):
        """psum->sbuf evacuation on ACT or DVE (GPSIMD cannot touch PSUM)."""
        eng = next(rot[stage])
        if eng == "a":
            nc.scalar.activation(dst, src, AF.Relu if relu else AF.Copy,
                                 scale=scale)
        else:
            if relu:
                nc.vector.tensor_scalar(dst, src, scale, 0.0, OP.mult, OP.max)
            elif scale == 1.0:
                nc.vector.tensor_copy(dst, src)
            else:
                nc.vector.tensor_scalar(dst, src, scale, None, OP.mult)

    with tile.TileContext(nc, pool_alloc_mode="queue") as tc, ExitStack() as ctx:
        perm = ctx.enter_context(tc.tile_pool(name="perm", bufs=1))
        wpool = ctx.enter_context(tc.tile_pool(name="wpool", bufs=1))

        ident8 = perm.tile([128, 128], FP8)
        make_identity(nc, ident8)
        ident_f = perm.tile([128, 128], F32)
        make_identity(nc, ident_f)

        warm = perm.tile([128, 1], F32)
        nc.scalar.activation(warm, ident_f[:, 0:1], AF.Exp)
        nc.scalar.activation(warm, ident_f[:, 0:1], AF.Abs)
        nc.scalar.activation(warm, ident_f[:, 0:1], AF.Ln, bias=1.0)

        # ---- weights + activations (persistent tiles) ----
        xT = wpool.tile([128, KC, R], FP8)
        fc1w = wpool.tile([128, KC, H], FP8)
        fc2w = wpool.tile([128, KC, H], FP8)
        wg = wpool.tile([128, KC, TE], FP8)
        ew1 = wpool.tile([128, E, KC, EH], FP8)
        ew2 = wpool.tile([128, E, KC, H], FP8)
        tw1 = wpool.tile([128, T, KC, TH], FP8)
        tw2 = wpool.tile([128, KC, T], FP8)

        # x shard on the SP hw queue; all weights on the ACT hw queue
        xTv = xT_d[:, :].rearrange("p (k r) -> p k r", k=KC)
        for (r0, r1) in RS:
            nc.sync.dma_start(xT[:, :, r0:r1], xTv[:, :, r0:r1])
        nc.scalar.dma_start(fc1w,
                            fc1w_d[:, :].rearrange("p (k h) -> p k h", k=KC))
        nc.scalar.dma_start(fc2w,
                            fc2w_d[:, :].rearrange("p (k h) -> p k h", k=KC))
        nc.scalar.dma_start(wg, wg_d[:, :].rearrange("p (k t) -> p k t", k=KC))
        nc.scalar.dma_start(
            ew1, ew1_d[:, :].rearrange("p (e k f) -> p e k f", e=E, k=KC))
        nc.scalar.dma_start(
            ew2, ew2_d[:, :].rearrange("p (e k f) -> p e k f", e=E, k=KC))
        nc.scalar.dma_start(
            tw1, tw1_d[:, :].rearrange("p (t k f) -> p t k f", t=T, k=KC))
        nc.scalar.dma_start(tw2, tw2_d[:, :].rearrange("p (k t) -> p k t", k=KC))

        scores_sb = perm.tile([BSH, T, C], F32)
        nc.sync.dma_start(scores_sb,
                          scores_d[:, :].rearrange("b (t c) -> b t c", t=T))
        srm_sb = perm.tile([128, NRT, T], F32)
        nc.sync.dma_start(srm_sb,
                          srm_d[:, :].rearrange("p (r t) -> p r t", r=NRT))
        sel_all = perm.tile([128, NRT, BSH], F32)
        nc.sync.dma_start(sel_all,
                          sel_d[:, :].rearrange("p (r b) -> p r b", r=NRT))
        selt_all = perm.tile([BSH, NRT, 128], F32)
        nc.sync.dma_start(selt_all,
                          selt_d[:, :].rearrange("b (r p) -> b r p", r=NRT))
        sel_sb = [sel_all[:, rt, :] for rt in range(NRT)]
        selt_sb = [selt_all[:, rt, :] for rt in range(NRT)]

        hT = perm.tile([128, KC, R], FP8)
        glog = perm.tile([128, NRT * TE], F32)
        gates = perm.tile([128, NRT * TE], F32)
        gates8 = perm.tile([128, NRT * TE], FP8)
        yT = perm.tile([128, T, KC, R], FP8)

        # chunked top-3-of-6 masked softmax on a [128, ng, E] glog slice
        gtmp = ctx.enter_context(tc.tile_pool(name="gtmp", bufs=1))

        def gating(lo, hi):
            en = nc.gpsimd if kn["gat"] == "p" else nc.vector
            ng = (hi - lo) * T
            v = glog[:, lo * TE:hi * TE].rearrange("p (g e) -> p g e", e=E)
            sh = [128, ng, E]
            m1 = gtmp.tile([128, NRT * T, 1], F32, name="m1", tag="m1",
                           bufs=2)[:, :ng, :]
            en.tensor_reduce(m1, v, AX.X, OP.max)
            m1b = m1.broadcast_to(sh)
            mask = gtmp.tile([128, NRT * T, E], F32, name="mk",
                             tag="mk", bufs=2)[:, :ng, :]
            en.tensor_tensor(mask, v, m1b, OP.is_ge)
            v2 = gtmp.tile([128, NRT * T, E], F32, name="v2", tag="v2",
                           bufs=2)[:, :ng, :]
            en.scalar_tensor_tensor(v2, mask, -1e30, v, OP.mult, OP.add)
            m2 = gtmp.tile([128, NRT * T, 1], F32, name="m2", tag="m2",
                           bufs=2)[:, :ng, :]
            en.tensor_reduce(m2, v2, AX.X, OP.max)
            en.tensor_tensor(mask, v2, m2.broadcast_to(sh), OP.is_ge)
            en.scalar_tensor_tensor(v2, mask, -1e30, v2, OP.mult, OP.add)
            en.tensor_reduce(m2, v2, AX.X, OP.max)
            keep = gtmp.tile([128, NRT * T, E], F32, name="kp", tag="kp",
                             bufs=2)[:, :ng, :]
            en.tensor_tensor(keep, v, m2.broadcast_to(sh), OP.is_ge)
            vs = gtmp.tile([128, NRT * T, E], F32, name="vs", tag="vs",
                           bufs=2)[:, :ng, :]
            en.tensor_tensor(vs, v, m1b, OP.subtract)
            ex = gtmp.tile([128, NRT * T, E], F32, name="ex", tag="ex",
                           bufs=2)[:, :ng, :]
            nc.scalar.activation(ex, vs, AF.Exp)
            en.tensor_tensor(ex, ex, keep, OP.mult)
            en.tensor_reduce(m1, ex, AX.X, OP.add)
            nc.vector.reciprocal(m1, m1)
            gv = gates[:, lo * TE:hi * TE].rearrange("p (g e) -> p g e", e=E)
            en.tensor_tensor(gv, ex, m1.broadcast_to(sh), OP.mult)
            for rt in range(lo, hi):
                # e-major fp8 copy: gates8[rt, e, t] = gates[rt, t, e]
                src = gates[:, rt * TE:(rt + 1) * TE] \
                    .rearrange("p (t e) -> p e t", e=E)
                dstv = gates8[:, rt * TE:(rt + 1) * TE] \
                    .rearrange("p (e t) -> p e t", t=T)
                en.tensor_copy(dstv, src)

        # gate diagonals (built on DVE as soon as each group's gates exist)
        diag_pool = ctx.enter_context(tc.tile_pool(name="diag", bufs=1))
        diag = {}


        def emit_diag(gi):
            t0, t1 = RG_RT[gi]
            for rt in range(t0, t1):
                dg = diag_pool.tile([128, E, T, 128], FP8,
                                    name=f"dg{rt}", tag="dg", bufs=15)
                g8 = gates8[:, rt * TE:(rt + 1) * TE] \
                    .rearrange("p (et f) -> p et f", f=1) \
                    .broadcast_to([128, TE, 128])
                i8 = ident8.rearrange("p (et f) -> p et f", et=1) \
                    .broadcast_to([128, TE, 128])
                dgv = dg.rearrange("p e t f -> p (e t) f")
                eng = next(rot["diag"])
                if eng == "p":
                    nc.gpsimd.affine_select(
                        out=dgv.rearrange("p et f -> p (et f)"), in_=g8,
                        pattern=[[0, TE], [-1, 128]],
                        compare_op=OP.is_equal, fill=0.0,
                        base=0, channel_multiplier=1)
                else:
                    nc.vector.tensor_tensor(dgv, i8, g8, OP.mult)
                diag[rt] = dg

        # ---------------- P1+P2: shared bottom, gate logits, gating -------
        with tc.tile_pool(name="p1sb", bufs=1) as p1sb, \
             tc.tile_pool(name="pA", bufs=6, space="PSUM") as pA, \
             tc.tile_pool(name="pG", bufs=1, space="PSUM") as pG:
            h1T = p1sb.tile([128, KC, R], FP8)
            pg = pG.tile([128, NRT, TE], F32, name="glog", tag="g")
            for (r0, r1) in RS:
                rw = r1 - r0
                for mc in range(KC):
                    pa = pA.tile([128, 512], F32, name="fc1", tag="acc")
                    for i in range(2):
                        nc.tensor.matmul(
                            pa[:, :rw],
                            fc1w[:, 2 * i:2 * i + 2, mc * 128:(mc + 1) * 128],
                            xT[:, 2 * i:2 * i + 2, r0:r1],
                            start=(i == 0), stop=(i == 1),
                            perf_mode=PM.DoubleRow)
                    ev("h", h1T[:, mc, r0:r1], pa[:, :rw], SC_H1, relu=True)
            for gi, (r0, r1) in enumerate(RS):
                rw = r1 - r0
                for mc in range(KC):
                    pa = pA.tile([128, 512], F32, name="fc2", tag="acc")
                    for i in range(2):
                        nc.tensor.matmul(
                            pa[:, :rw],
                            fc2w[:, 2 * i:2 * i + 2, mc * 128:(mc + 1) * 128],
                            h1T[:, 2 * i:2 * i + 2, r0:r1],
                            start=(i == 0), stop=(i == 1),
                            perf_mode=PM.DoubleRow)
                    ev("h", hT[:, mc, r0:r1], pa[:, :rw], SC_H, relu=False)
                t0, t1 = RG_RT[gi]
                for rt in range(t0, t1):
                    for i in range(2):
                        nc.tensor.matmul(
                            pg[:, rt, :],
                            hT[:, 2 * i:2 * i + 2, rt * 128:(rt + 1) * 128],
                            wg[:, 2 * i:2 * i + 2, :],
                            start=(i == 0), stop=(i == 1),
                            perf_mode=PM.DoubleRow)
                nc.scalar.activation(glog[:, t0 * TE:t1 * TE],
                                     pg[:, t0:t1, :], AF.Copy, scale=SC_GLOG)
                gating(t0, t1)
                emit_diag(gi)

        # ---------------- P3: experts + gated combine into yT -------------
        with tc.tile_pool(name="exp", bufs=1) as exp, \
             tc.tile_pool(name="psE", bufs=3, space="PSUM") as psE, \
             tc.tile_pool(name="psO", bufs=3, space="PSUM") as psO, \
             tc.tile_pool(name="psY", bufs=2, space="PSUM") as psY:
            eo_t = {}

            def ehs_tile_emitter(gi, pr):
                """yields the 8 (psum-tile mm + evac) chunks of an ehs stage"""
                r0, r1 = RS[gi]
                rgw = r1 - r0
                ehs2 = exp.tile([128, 2, KC, 512], FP8, name="ehs",
                                tag="ehs", bufs=3)

                def chunk(j, mc):
                    e = 2 * pr + j
                    ps = psE.tile([128, 512], F32, name="ehs", tag="ehs")
                    for i in range(2):
                        nc.tensor.matmul(
                            ps[:, :rgw],
                            ew1[:, e, 2 * i:2 * i + 2, mc * 128:(mc + 1) * 128],
                            hT[:, 2 * i:2 * i + 2, r0:r1],
                            start=(i == 0), stop=(i == 1),
                            perf_mode=PM.DoubleRow)
                    ev("ehs" if gi < 2 else "ehs2",
                       ehs2[:, j, mc, :rgw], ps[:, :rgw], SC_EHS, relu=True)

                return ehs2, [(chunk, j, mc)
                              for j in range(2) for mc in range(KC)]

            def eo_chunks(gi, pr, ehs2):
                """yields per-(row-tile, expert) eo mm + evac chunks"""
                t0, t1 = RG_RT[gi]

                def chunk(rtl, j):
                    rt = t0 + rtl
                    if pr == 0 and j == 0:
                        eo_t[rt] = exp.tile([128, E, H], FP8, name=f"eo{rt}",
                                            tag="eo", bufs=8)
                    po = psO.tile([128, 512], F32, name="eo", tag="eo")
                    for i in range(2):
                        nc.tensor.matmul(
                            po,
                            ehs2[:, j, 2 * i:2 * i + 2,
                                 rtl * 128:(rtl + 1) * 128],
                            ew2[:, 2 * pr + j, 2 * i:2 * i + 2, :],
                            start=(i == 0), stop=(i == 1),
                            perf_mode=PM.DoubleRow)
                    ev("eo" if gi < 2 else "eo2",
                       eo_t[rt][:, 2 * pr + j, :], po, SC_EO)

                return [(chunk, rtl, j)
                        for rtl in range(t1 - t0) for j in range(2)]

            def yt_chunks(gi):
                """gated-combine chunks for a finished group (yT psums share
                the psE tag so they interleave with the next stage's ehs)"""
                t0, t1 = RG_RT[gi]

                def chunk(rtl, hc):
                    rt = t0 + rtl
                    py = psY.tile([128, 512], F32, name="yt", tag="yt")
                    for p2 in range(3):
                        nc.tensor.matmul(
                            py[:, 0:T * 128],
                            eo_t[rt][:, 2 * p2:2 * p2 + 2,
                                     hc * 128:(hc + 1) * 128],
                            diag[rt][:, 2 * p2:2 * p2 + 2, :, :],
                            start=(p2 == 0), stop=(p2 == 2),
                            perf_mode=PM.DoubleRow)
                    src = py[:, 0:T * 128].rearrange(
                        "p (t r) -> p t r", t=T)
                    dst = yT[:, :, hc, rt * 128:(rt + 1) * 128]
                    ev("yT", dst, src, SC_YT)

                return [(chunk, rtl, hc)
                        for rtl in range(t1 - t0) for hc in range(KC)]

            def run_interleaved(a, b):
                """emit chunks of list a (new stage, may stall on psum reuse)
                interleaved evenly with ready chunks of list b (prev stage)"""
                na, nb = len(a), len(b)
                order = []
                bi = 0
                for ai in range(na):
                    order.append(a[ai])
                    want = (ai + 1) * nb // max(na, 1)
                    while bi < want:
                        order.append(b[bi])
                        bi += 1
                order.extend(b[bi:])
                for chunk, *args in order:
                    chunk(*args)

            # software pipeline over (group, expert-pair) stages:
            # ehs(stage k+1) emission interleaved with eo/yT of stage k.
            stages = [(gi, pr) for gi in range(len(RS)) for pr in range(3)]
            pending = []    # ready chunks from the previous stage
            for gi, pr in stages:
                ehs2, achunks = ehs_tile_emitter(gi, pr)
                run_interleaved(achunks, pending)
                pending = eo_chunks(gi, pr, ehs2)
                if pr == 2:
                    pending = pending + yt_chunks(gi)
            run_interleaved([], pending)

        # ---------------- P4: towers + logits + labels + aux --------------
        bce = ctx.enter_context(tc.tile_pool(name="bce", bufs=1))
        with tc.tile_pool(name="tow", bufs=1) as tow, \
             tc.tile_pool(name="psT", bufs=4, space="PSUM") as psT, \
             tc.tile_pool(name="psL", bufs=1, space="PSUM") as psL, \
             tc.tile_pool(name="psS", bufs=1, space="PSUM") as psS:
            pl = psL.tile([128, NRT, T], F32, name="lg", tag="lg")

            # labels: per-task max score broadcast to rows, compare
            smax3 = bce.tile([BSH, T, 1], F32)
            nc.vector.tensor_reduce(smax3, scores_sb, AX.X, OP.max)
            smax = bce.tile([BSH, T], F32)
            nc.vector.tensor_copy(smax, smax3.rearrange("b t o -> b (t o)"))
            plab = psS.tile([128, NRT, T], F32, name="lab", tag="lab")
            for rt in range(NRT):
                nc.tensor.matmul(plab[:, rt, :], selt_sb[rt], smax,
                                 start=True, stop=True)
            labels = bce.tile([128, NRT, T], F32)
            nc.vector.tensor_tensor(labels, srm_sb, plab, OP.is_equal)

            # aux: imp[te,b] = sum_rows gates * sel  (f32 matmuls)
            pim = psS.tile([TE, BSH], F32, name="imp", tag="tiny")
            for rt in range(NRT):
                nc.tensor.matmul(pim, gates[:, rt * TE:(rt + 1) * TE],
                                 sel_sb[rt], start=(rt == 0),
                                 stop=(rt == NRT - 1))
            imp_sb = bce.tile([TE, BSH], F32)
            nc.vector.tensor_copy(imp_sb, pim)
            pit = psS.tile([BSH, TE], F32, name="impT", tag="tiny")
            nc.tensor.transpose(pit, imp_sb, ident_f[:TE, :TE])
            impT = bce.tile([BSH, TE], F32)
            nc.vector.tensor_copy(impT, pit)
            impTv = impT.rearrange("b (t e) -> b t e", e=E)
            auxs = bce.tile([BSH, 1], F32)
            for t in range(T):
                st = bce.tile([BSH, 6], F32, name=f"bnst{t}")
                nc.vector.bn_stats(st, impTv[:, t, :])
                mv = bce.tile([BSH, 2], F32, name=f"bnmv{t}")
                nc.vector.bn_aggr(mv, st)
                msq = bce.tile([BSH, 1], F32, name=f"msq{t}")
                nc.vector.tensor_tensor(msq, mv[:, 0:1], mv[:, 0:1], OP.mult)
                nc.vector.tensor_scalar(msq, msq, 1e-10, None, OP.add)
                rec = bce.tile([BSH, 1], F32, name=f"rec{t}")
                nc.vector.reciprocal(rec, msq)
                cv2 = bce.tile([BSH, 1], F32, name=f"cv2{t}")
                nc.vector.tensor_tensor(cv2, mv[:, 1:2], rec, OP.mult)
                if t == 0:
                    nc.vector.tensor_copy(auxs, cv2)
                else:
                    nc.vector.tensor_tensor(auxs, auxs, cv2, OP.add)

            thT_t = {}

            def tower_chunks(t):
                thT = tow.tile([128, KC, R], FP8, name=f"th{t}", tag="th",
                               bufs=2)
                thT_t[t] = thT

                def chunk(r0, r1, mc):
                    rw = r1 - r0
                    ps = psT.tile([128, 512], F32, name="th", tag="th")
                    for i in range(2):
                        nc.tensor.matmul(
                            ps[:, :rw],
                            tw1[:, t, 2 * i:2 * i + 2, mc * 128:(mc + 1) * 128],
                            yT[:, t, 2 * i:2 * i + 2, r0:r1],
                            start=(i == 0), stop=(i == 1),
                            perf_mode=PM.DoubleRow)
                    ev("thT", thT[:, mc, r0:r1], ps[:, :rw], SC_TH, relu=True)

                return [(chunk, r0, r1, mc)
                        for (r0, r1) in RS for mc in range(KC)]

            def logit_chunks(t):
                def chunk(rt):
                    for i in range(2):
                        nc.tensor.matmul(
                            pl[:, rt, t:t + 1],
                            thT_t[t][:, 2 * i:2 * i + 2,
                                     rt * 128:(rt + 1) * 128],
                            tw2[:, 2 * i:2 * i + 2, t:t + 1],
                            start=(i == 0), stop=(i == 1),
                            perf_mode=PM.DoubleRow)
                return [(chunk, rt) for rt in range(NRT)]

            pend = []
            for t in range(T):
                a = tower_chunks(t)
                na, nb = len(a), len(pend)
                order, bi = [], 0
                for ai in range(na):
                    order.append(a[ai])
                    want = (ai + 1) * nb // na
                    while bi < want:
                        order.append(pend[bi])
                        bi += 1
                order.extend(pend[bi:])
                for chunk, *args in order:
                    chunk(*args)
                pend = logit_chunks(t)
            for chunk, *args in pend:
                chunk(*args)
            logits = bce.tile([128, NRT, T], F32)
            nc.scalar.activation(logits, pl, AF.Copy, scale=SC_LOG)

            # ---------------- P5: BCE + reduction ----------------
            t1_ = bce.tile([128, NRT, T], F32)
            nc.vector.tensor_scalar(t1_, logits, 0.0, None, OP.max)
            t2_ = bce.tile([128, NRT, T], F32)
            nc.vector.tensor_tensor(t2_, logits, labels, OP.mult)
            absl = bce.tile([128, NRT, T], F32)
            nc.scalar.activation(absl, logits, AF.Abs)
            expl = bce.tile([128, NRT, T], F32)
            nc.scalar.activation(expl, absl, AF.Exp, scale=-1.0)
            lp = bce.tile([128, NRT, T], F32)
            nc.scalar.activation(lp, expl, AF.Ln, bias=1.0)
            nc.vector.tensor_tensor(t1_, t1_, t2_, OP.subtract)
            nc.vector.tensor_tensor(t1_, t1_, lp, OP.add)
            bs = bce.tile([128, NRT], F32)
            nc.vector.tensor_reduce(bs, t1_, AX.X, OP.add)
            pb = psS.tile([BSH, 1], F32, name="bsum", tag="tiny")
            for rt in range(NRT):
                nc.tensor.matmul(pb, sel_sb[rt], bs[:, rt:rt + 1],
                                 start=(rt == 0), stop=(rt == NRT - 1))
            tsum = bce.tile([BSH, 1], F32)
            nc.vector.tensor_copy(tsum, pb)

            loss_sb = bce.tile([BSH, 1], F32)
            nc.vector.tensor_scalar(loss_sb, tsum, 1.0 / (T * C), None, OP.mult)
            auxf = bce.tile([BSH, 1], F32)
            nc.vector.tensor_scalar(auxf, auxs, LOSS_COEF, None, OP.mult)
            nc.vector.tensor_tensor(loss_sb, loss_sb, auxf, OP.add)
            nc.sync.dma_start(loss_d[:, :], loss_sb)

    nc.compile()
    return nc


_SEL_CACHE = None


def sel_mats():
    """0/1 selector matrices mapping rows r=rt*128+p to samples b=r//30."""
    global _SEL_CACHE
    if _SEL_CACHE is None:
        sel = np.zeros((NRT, 128, BSH), np.float32)
        for rt in range(NRT):
            for p in range(128):
                b = (rt * 128 + p) // C
                sel[rt, p, b] = 1.0
        selt = np.ascontiguousarray(sel.transpose(0, 2, 1))
        _SEL_CACHE = (sel, selt)
    return _SEL_CACHE


def _kchunk(w):
    """[K, N] f32 -> [128, KC, N] fp8 (pre-scaled)."""
    kdim = w.shape[0]
    return np.ascontiguousarray(
        (w * WS).reshape(kdim // 128, 128, -1).transpose(1, 0, 2)).astype(E4)


def host_prep_fp8(inputs):
    x = np.asarray(inputs["candidate_cls_embed"], np.float32)
    scores = np.asarray(inputs["scores"], np.float32)
    fc1_w = np.asarray(inputs["fc1_w"], np.float32)
    fc2_w = np.asarray(inputs["fc2_w"], np.float32)
    w_gate = np.asarray(inputs["w_gate"], np.float32)
    ew1 = np.asarray(inputs["expert_w1"], np.float32)
    ew2 = np.asarray(inputs["expert_w2"], np.float32)
    tw1 = np.asarray(inputs["tower_w1"], np.float32)
    tw2 = np.asarray(inputs["tower_w2"], np.float32)

    wgm = np.ascontiguousarray(w_gate.transpose(1, 0, 2)).reshape(H, TE)
    ew1m = (ew1 * WS).reshape(E, KC, 128, EH).transpose(2, 0, 1, 3)
    ew2m = (ew2 * WS).reshape(E, KC, 128, H).transpose(2, 0, 1, 3)
    tw1m = (tw1 * WS).reshape(T, KC, 128, TH).transpose(2, 0, 1, 3)
    tw2m = (tw2.T * WS).reshape(KC, 128, T).transpose(1, 0, 2)

    sel, selt = sel_mats()
    shared = {
        "fc1w": _kchunk(fc1_w).reshape(128, KC * H),
        "fc2w": _kchunk(fc2_w).reshape(128, KC * H),
        "wg": _kchunk(wgm).reshape(128, KC * TE),
        "ew1": np.ascontiguousarray(ew1m).astype(E4).reshape(128, -1),
        "ew2": np.ascontiguousarray(ew2m).astype(E4).reshape(128, -1),
        "tw1": np.ascontiguousarray(tw1m).astype(E4).reshape(128, -1),
        "tw2": np.ascontiguousarray(tw2m).astype(E4).reshape(128, -1),
        "sel": np.ascontiguousarray(sel.transpose(1, 0, 2)).reshape(128, -1),
        "selt": np.ascontiguousarray(selt.transpose(1, 0, 2)).reshape(BSH, -1),
    }
    in_maps = []
    for ci in range(NCORES):
        xs = x[ci * BSH:(ci + 1) * BSH].reshape(R, H)
        xTc = np.ascontiguousarray(xs.T.reshape(KC, 128, R).transpose(1, 0, 2))
        m = dict(shared)
        m["xT"] = xTc.astype(E4).reshape(128, KC * R)
        sc = np.ascontiguousarray(scores[ci * BSH:(ci + 1) * BSH])
        m["scores"] = sc.reshape(BSH, T * C)
        srm = sc.transpose(0, 2, 1).reshape(NRT, 128, T).transpose(1, 0, 2)
        m["srm"] = np.ascontiguousarray(srm).reshape(128, NRT * T)
        in_maps.append(m)
    return in_maps


# ---------------------------------------------------------------------------
# bf16 fallback (non-zero biases)
# ---------------------------------------------------------------------------
import numpy as np
import ml_dtypes

import concourse.bass as bass
import concourse.tile as tile
from concourse import bacc, mybir
from concourse.masks import make_identity
from concourse.bass_utils import run_bass_kernel_spmd

F32 = mybir.dt.float32
BF16 = mybir.dt.bfloat16
BF = ml_dtypes.bfloat16
AF = mybir.ActivationFunctionType
OP = mybir.AluOpType
AX = mybir.AxisListType

NCORES = 8
B, C, T, H, E, EH, TH = 512, 30, 3, 512, 6, 512, 512
BSH = B // NCORES          # 64 samples per core
R = BSH * C                # 1920 rows per core
NRT = R // 128             # 15 row tiles
KC = H // 128              # 4 feature chunks
RS = [(0, 512), (512, 1024), (1024, 1536), (1536, R)]  # row slices (<=512)
RG_RT = [(0, 4), (4, 8), (8, 12), (12, 15)]            # row tiles per group
LOSS_COEF = 0.01

# engine-assignment knobs (tuned against the timeline cost model)
KNOBS_BF16 = {
    "eo_copy_dve_every": 2,   # every k-th eo psum->sbuf copy goes to DVE
    "ts_pool_every": 4,       # every k-th gate-scale mult on GpSimd
    "add_pool_every": 5,      # every k-th y-accumulate add goes to GpSimd
    "ytr_copy_act_every": 2,  # every k-th y-transpose psum->sbuf copy on ACT
}

_CACHED_BF16 = {}


def build_nc_bf16(zero_bias: bool):
    nc = bacc.Bacc(None, target_bir_lowering=False, debug=False)

    xT_d = nc.dram_tensor("xT", [KC, 128, R], BF16, kind="ExternalInput")
    scores_d = nc.dram_tensor("scores", [BSH, T, C], F32, kind="ExternalInput")
    fc1w_d = nc.dram_tensor("fc1w", [KC, 128, H], BF16, kind="ExternalInput")
    fc1b_d = nc.dram_tensor("fc1b", [128, KC], F32, kind="ExternalInput")
    fc2w_d = nc.dram_tensor("fc2w", [KC, 128, H], BF16, kind="ExternalInput")
    fc2b_d = nc.dram_tensor("fc2b", [128, KC], F32, kind="ExternalInput")
    wg_d = nc.dram_tensor("wg", [KC, 128, T * E], BF16, kind="ExternalInput")
    ew1_d = nc.dram_tensor("ew1", [E, KC, 128, EH], BF16, kind="ExternalInput")
    eb1_d = nc.dram_tensor("eb1", [E, 128, KC], F32, kind="ExternalInput")
    ew2_d = nc.dram_tensor("ew2", [E, KC, 128, H], BF16, kind="ExternalInput")
    eb2_d = nc.dram_tensor("eb2", [E, 1, H], BF16, kind="ExternalInput")
    tw1_d = nc.dram_tensor("tw1", [T, KC, 128, TH], BF16, kind="ExternalInput")
    tb1_d = nc.dram_tensor("tb1", [T, 128, KC], F32, kind="ExternalInput")
    tw2_d = nc.dram_tensor("tw2", [T, 128, KC], BF16, kind="ExternalInput")
    tb2_d = nc.dram_tensor("tb2", [128, T], F32, kind="ExternalInput")
    sel_d = nc.dram_tensor("sel", [NRT, 128, BSH], F32, kind="ExternalInput")
    selt_d = nc.dram_tensor("selt", [NRT, BSH, 128], F32, kind="ExternalInput")
    srm_d = nc.dram_tensor("srm", [128, NRT, T], F32, kind="ExternalInput")
    loss_d = nc.dram_tensor("loss", [BSH, 1], F32, kind="ExternalOutput")

    eo_dve = KNOBS_BF16["eo_copy_dve_every"]
    ts_pool = KNOBS_BF16.get("ts_pool_every", 5)
    add_pool = KNOBS_BF16["add_pool_every"]
    ytr_act = KNOBS_BF16["ytr_copy_act_every"]

    with tile.TileContext(nc, pool_alloc_mode="queue") as tc, ExitStack() as ctx:
        perm = ctx.enter_context(tc.tile_pool(name="perm", bufs=1))
        dram = ctx.enter_context(tc.tile_pool(name="dram", bufs=1, space="DRAM"))
        psA = ctx.enter_context(tc.tile_pool(name="psA", bufs=5, space="PSUM"))
        psB = ctx.enter_context(tc.tile_pool(name="psB", bufs=2, space="PSUM"))
        hpool = ctx.enter_context(tc.tile_pool(name="hpool", bufs=1))

        ident_bf = perm.tile([128, 128], BF16)
        make_identity(nc, ident_bf)
        ident_f = perm.tile([128, 128], F32)
        make_identity(nc, ident_f)
        if not zero_bias:
            ones_bf = perm.tile([1, 128], BF16)
            nc.vector.memset(ones_bf, 1.0)

        warm = perm.tile([128, 1], F32)
        nc.scalar.activation(warm, ident_f[:, 0:1], AF.Exp)
        nc.scalar.activation(warm, ident_f[:, 0:1], AF.Abs)
        nc.scalar.activation(warm, ident_f[:, 0:1], AF.Ln, bias=1.0)

        scores_sb = perm.tile([BSH, T, C], F32)
        nc.sync.dma_start(scores_sb, scores_d[:, :, :])
        srm_sb = perm.tile([128, NRT, T], F32)
        sel_sb = [perm.tile([128, BSH], F32, name=f"sel{rt}") for rt in range(NRT)]
        selt_sb = [perm.tile([BSH, 128], F32, name=f"selt{rt}") for rt in range(NRT)]
        if not zero_bias:
            tb2_sb = perm.tile([128, 1, T], F32)
            nc.sync.dma_start(tb2_sb, tb2_d[:, :])

        glog = perm.tile([128, NRT * T * E], F32)    # [128, 270] row-major
        gates = perm.tile([128, NRT * T * E], F32)
        gates_fm = perm.tile([T * E, R], F32)        # [18, 1920] feature-major
        ypool = ctx.enter_context(tc.tile_pool(name="ypool", bufs=1))
        yT = [ypool.tile([128, KC * R], BF16, name=f"yT{t}") for t in range(T)]
        logits_sb = perm.tile([128, NRT, T], F32)

        hT = [hpool.tile([128, R], BF16, name=f"hT{k}") for k in range(KC)]

        # expert weights: resident for the whole expert phase
        epool = ctx.enter_context(tc.tile_pool(name="epool", bufs=1))

        # ---------------- phase 1+2: shared bottom ----------------
        with tc.tile_pool(name="early", bufs=1) as early:
            fc1w = [early.tile([128, H], BF16, name=f"fc1w{k}")
                    for k in range(KC)]
            fc2w = [early.tile([128, H], BF16, name=f"fc2w{k}")
                    for k in range(KC)]
            wgw = [early.tile([128, T * E], BF16, name=f"wg{k}")
                   for k in range(KC)]
            xT = [early.tile([128, R], BF16, name=f"xT{k}") for k in range(KC)]
            r0, r1 = RS[0]
            for k in range(KC):
                nc.sync.dma_start(fc1w[k], fc1w_d[k, :, :])
                nc.sync.dma_start(xT[k][:, r0:r1], xT_d[k, :, r0:r1])
            for k in range(KC):
                nc.sync.dma_start(wgw[k], wg_d[k, :, :])
                nc.sync.dma_start(fc2w[k], fc2w_d[k, :, :])
            for (r0, r1) in RS[1:]:
                for k in range(KC):
                    nc.sync.dma_start(xT[k][:, r0:r1], xT_d[k, :, r0:r1])
            if not zero_bias:
                fc1b = early.tile([128, KC], F32)
                nc.sync.dma_start(fc1b, fc1b_d[:, :])
                fc2b = early.tile([128, KC], F32)
                nc.sync.dma_start(fc2b, fc2b_d[:, :])

            ew1 = [[None] * KC for _ in range(E)]
            ew2 = [[None] * KC for _ in range(E)]
            eb1 = [None] * E
            eb2row = [None] * E
            for e in range(E):
                for k in range(KC):
                    w1 = epool.tile([128, EH], BF16, name=f"ew1_{e}_{k}")
                    nc.sync.dma_start(w1, ew1_d[e, k, :, :])
                    ew1[e][k] = w1
                    w2 = epool.tile([128, H], BF16, name=f"ew2_{e}_{k}")
                    nc.sync.dma_start(w2, ew2_d[e, k, :, :])
                    ew2[e][k] = w2
                if not zero_bias:
                    b1 = epool.tile([128, KC], F32, name=f"eb1_{e}")
                    nc.sync.dma_start(b1, eb1_d[e, :, :])
                    eb1[e] = b1
                    b2r = epool.tile([1, H], BF16, name=f"eb2_{e}")
                    nc.sync.dma_start(b2r, eb2_d[e, :, :])
                    eb2row[e] = b2r

            nc.sync.dma_start(srm_sb, srm_d[:, :, :])
            for rt in range(NRT):
                nc.sync.dma_start(sel_sb[rt], sel_d[rt, :, :])
                nc.sync.dma_start(selt_sb[rt], selt_d[rt, :, :])

            h1T = [early.tile([128, R], BF16, name=f"h1T{k}") for k in range(KC)]
            for mc in range(KC):
                for (r0, r1) in RS:
                    ps = psA.tile([128, r1 - r0], F32, name="accB", tag="acc")
                    for k in range(KC):
                        nc.tensor.matmul(
                            ps, fc1w[k][:, mc * 128:(mc + 1) * 128], xT[k][:, r0:r1],
                            start=(k == 0), stop=(k == KC - 1))
                    if zero_bias:
                        nc.scalar.activation(h1T[mc][:, r0:r1], ps, AF.Relu)
                    else:
                        nc.scalar.activation(h1T[mc][:, r0:r1], ps, AF.Relu,
                                             bias=fc1b[:, mc:mc + 1])
            for mc in range(KC):
                for (r0, r1) in RS:
                    ps = psA.tile([128, r1 - r0], F32, name="accB2", tag="acc")
                    for k in range(KC):
                        nc.tensor.matmul(
                            ps, fc2w[k][:, mc * 128:(mc + 1) * 128], h1T[k][:, r0:r1],
                            start=(k == 0), stop=(k == KC - 1))
                    if zero_bias:
                        nc.scalar.activation(hT[mc][:, r0:r1], ps, AF.Copy)
                    else:
                        nc.scalar.activation(hT[mc][:, r0:r1], ps, AF.Identity,
                                             bias=fc2b[:, mc:mc + 1])

            # ---------------- phase 3: gate logits (row-major) ----------------
            GE = T * E
            for rt in range(NRT):
                ps = psA.tile([128, GE], F32, name="accG", tag="acc")
                for k in range(KC):
                    nc.tensor.matmul(
                        ps, hT[k][:, rt * 128:(rt + 1) * 128], wgw[k],
                        start=(k == 0), stop=(k == KC - 1))
                nc.scalar.activation(glog[:, rt * GE:(rt + 1) * GE], ps, AF.Copy)

        # ---------------- gating: top-3-of-6 masked softmax ----------------
        NG = NRT * T  # 45 groups of E
        v = glog.rearrange("p (g e) -> p g e", e=E)
        gtmp = ctx.enter_context(tc.tile_pool(name="gtmp", bufs=1))  # noqa
        neginf = gtmp.tile([128, NG, E], F32)
        nc.vector.memset(neginf, -1e30)
        m1 = gtmp.tile([128, NG, 1], F32)
        nc.vector.tensor_reduce(m1, v, AX.X, OP.max)
        m1b = m1.broadcast_to([128, NG, E])
        mask = gtmp.tile([128, NG, E], mybir.dt.uint8)
        nc.vector.tensor_tensor(mask, v, m1b, OP.is_ge)
        v2 = gtmp.tile([128, NG, E], F32)
        nc.vector.select(v2, mask, neginf, v)
        m2 = gtmp.tile([128, NG, 1], F32)
        nc.vector.tensor_reduce(m2, v2, AX.X, OP.max)
        mask2 = gtmp.tile([128, NG, E], mybir.dt.uint8)
        nc.vector.tensor_tensor(mask2, v2, m2.broadcast_to([128, NG, E]), OP.is_ge)
        v3 = gtmp.tile([128, NG, E], F32)
        nc.vector.select(v3, mask2, neginf, v2)
        m3 = gtmp.tile([128, NG, 1], F32)
        nc.vector.tensor_reduce(m3, v3, AX.X, OP.max)
        keep = gtmp.tile([128, NG, E], F32)
        nc.vector.tensor_tensor(keep, v, m3.broadcast_to([128, NG, E]), OP.is_ge)
        vs = gtmp.tile([128, NG, E], F32)
        nc.vector.tensor_tensor(vs, v, m1b, OP.subtract)
        ex = gtmp.tile([128, NG, E], F32)
        nc.scalar.activation(ex, vs, AF.Exp)
        ek = gtmp.tile([128, NG, E], F32)
        nc.vector.tensor_tensor(ek, ex, keep, OP.mult)
        ssum = gtmp.tile([128, NG, 1], F32)
        nc.vector.tensor_reduce(ssum, ek, AX.X, OP.add)
        rsum = gtmp.tile([128, NG, 1], F32)
        nc.vector.reciprocal(rsum, ssum)
        gv = gates.rearrange("p (g e) -> p g e", e=E)
        nc.vector.tensor_tensor(gv, ek, rsum.broadcast_to([128, NG, E]), OP.mult)

        # gates feature-major (for aux loss): PE transpose per row tile
        GE = T * E
        for rt in range(NRT):
            gp = psB.tile([GE, 128], F32, name="gtr", tag="small", bufs=1)
            nc.tensor.transpose(gp, gates[:, rt * GE:(rt + 1) * GE], ident_f)
            nc.vector.tensor_copy(gates_fm[:, rt * 128:(rt + 1) * 128], gp)

        # aux: imp[t,e,b] = sum_c gates_fm -> cv^2 per (b,t)
        imp = perm.tile([T * E, BSH], F32)
        nc.vector.tensor_reduce(
            imp, gates_fm.rearrange("p (b c) -> p b c", c=C), AX.X, OP.add)
        ip = psB.tile([BSH, T * E], F32, name="itr", tag="small", bufs=1)
        nc.tensor.transpose(ip, imp, ident_f[:T * E, :T * E])
        impT = perm.tile([BSH, T * E], F32)
        nc.vector.tensor_copy(impT, ip)
        impTv = impT.rearrange("b (t e) -> b t e", e=E)
        auxs = perm.tile([BSH, 1], F32)
        for t in range(T):
            st = perm.tile([BSH, 6], F32, name=f"bnst{t}")
            nc.vector.bn_stats(st, impTv[:, t, :])
            mv = perm.tile([BSH, 2], F32, name=f"bnmv{t}")
            nc.vector.bn_aggr(mv, st)
            msq = perm.tile([BSH, 1], F32, name=f"msq{t}")
            nc.vector.tensor_tensor(msq, mv[:, 0:1], mv[:, 0:1], OP.mult)
            nc.vector.tensor_scalar(msq, msq, 1e-10, None, OP.add)
            rec = perm.tile([BSH, 1], F32, name=f"rec{t}")
            nc.vector.reciprocal(rec, msq)
            cv2 = perm.tile([BSH, 1], F32, name=f"cv2{t}")
            nc.vector.tensor_tensor(cv2, mv[:, 1:2], rec, OP.mult)
            if t == 0:
                nc.vector.tensor_copy(auxs, cv2)
            else:
                nc.vector.tensor_tensor(auxs, auxs, cv2, OP.add)

        # ------------- phase 4: experts, row-group blocked -------------
        nco = 0  # rotating index for engine-split knobs
        _st = {"n": 0}

        def emit_transposes(rg):
            pt0, pt1, pyg = rg
            for rtl in range(pt1 - pt0):
                rt = pt0 + rtl
                for t in range(T):
                    tp = psB.tile([128, KC, 128], BF16, name="ytr", tag="tr",
                                  bufs=2)
                    for jc in range(KC):
                        nc.tensor.transpose(
                            tp[:, jc, :], pyg[t][rtl][:, jc * 128:(jc + 1) * 128],
                            ident_bf)
                    dst = bass.AP(
                        tensor=yT[t].tensor, offset=yT[t].offset + rt * 128,
                        ap=[yT[t].ap[0], [R, KC], [1, 128]])
                    _st["n"] += 1
                    if _st["n"] % ytr_act == 0:
                        nc.scalar.activation(dst, tp, AF.Copy)
                    else:
                        nc.vector.tensor_copy(dst, tp)

        prev_rg = None
        with tc.tile_pool(name="exp", bufs=2) as exp:
            for gi, ((r0, r1), (t0, t1)) in enumerate(zip(RS, RG_RT)):
                rgw = r1 - r0
                yg = [[exp.tile([128, H], BF16, name=f"yg{t}_{rtl}", tag="yg",
                                bufs=20) for rtl in range(t1 - t0)]
                      for t in range(T)]
                for e in range(E):
                    if e == 2 and prev_rg is not None:
                        emit_transposes(prev_rg)
                        prev_rg = None
                    ehs = [exp.tile([128, rgw], BF16, name=f"ehs{k}",
                                    tag=f"ehs{k}") for k in range(KC)]
                    for mc in range(KC):
                        ps = psA.tile([128, rgw], F32, name="accE", tag="acc")
                        for k in range(KC):
                            nc.tensor.matmul(
                                ps, ew1[e][k][:, mc * 128:(mc + 1) * 128],
                                hT[k][:, r0:r1],
                                start=(k == 0), stop=(k == KC - 1))
                        if zero_bias:
                            nc.scalar.activation(ehs[mc], ps, AF.Relu)
                        else:
                            nc.scalar.activation(ehs[mc], ps, AF.Relu,
                                                 bias=eb1[e][:, mc:mc + 1])
                    for rtl in range(t1 - t0):
                        rt = t0 + rtl
                        ps = psA.tile([128, H], F32, name="accO", tag="acc")
                        for k in range(KC):
                            nc.tensor.matmul(
                                ps, ehs[k][:, rtl * 128:(rtl + 1) * 128], ew2[e][k],
                                start=(k == 0),
                                stop=(k == KC - 1) and zero_bias)
                        if not zero_bias:
                            nc.tensor.matmul(ps, ones_bf, eb2row[e],
                                             start=False, stop=True)
                        # evacuate eo once; combine from SBUF bf16 (fast modes)
                        eo = exp.tile([128, H], BF16, name="eo", tag="eo",
                                      bufs=6)
                        nco += 1
                        if (nco % eo_dve == 0) if eo_dve > 0 else (nco % -eo_dve != 0):
                            nc.vector.tensor_copy(eo, ps)
                        else:
                            nc.scalar.activation(eo, ps, AF.Copy)
                        for t in range(T):
                            g_ap = gates[:, rt * 18 + t * 6 + e:
                                         rt * 18 + t * 6 + e + 1]
                            nco += 1
                            if e == 0:
                                nc.vector.tensor_scalar(
                                    yg[t][rtl], eo, g_ap, None, OP.mult)
                            else:
                                tmp = exp.tile([128, H], BF16, name="ysc",
                                               tag="ysc", bufs=4)
                                if nco % ts_pool == 0:
                                    nc.gpsimd.tensor_scalar(tmp, eo, g_ap,
                                                            None, OP.mult)
                                else:
                                    nc.vector.tensor_scalar(tmp, eo, g_ap,
                                                            None, OP.mult)
                                if nco % add_pool == 0:
                                    nc.gpsimd.tensor_tensor(
                                        yg[t][rtl], yg[t][rtl], tmp, OP.add)
                                else:
                                    nc.vector.tensor_tensor(
                                        yg[t][rtl], yg[t][rtl], tmp, OP.add)
                prev_rg = (t0, t1, yg)
            emit_transposes(prev_rg)

        # labels in row-major layout: smax -> broadcast (selector matmuls)
        smax = perm.tile([BSH, T], F32)
        smax3 = perm.tile([BSH, T, 1], F32)
        nc.vector.tensor_reduce(smax3, scores_sb, AX.X, OP.max)
        nc.vector.tensor_copy(smax, smax3.rearrange("b t one -> b (t one)"))
        smax_bc = perm.tile([128, NRT, T], F32)
        for rt in range(NRT):
            pb = psB.tile([128, T], F32, name="smb", tag="small", bufs=1)
            nc.tensor.matmul(pb, selt_sb[rt], smax, start=True, stop=True)
            nc.vector.tensor_copy(smax_bc[:, rt, :], pb)
        labels_rm = perm.tile([128, NRT, T], F32)
        nc.vector.tensor_tensor(labels_rm, srm_sb, smax_bc, OP.is_equal)

        # ---------------- phase 5: towers ----------------
        with tc.tile_pool(name="tow", bufs=2) as tow:
            for t in range(T):
                tw1 = []
                for k in range(KC):
                    w1 = tow.tile([128, TH], BF16, name=f"tw1_{k}", tag=f"tw1_{k}")
                    nc.sync.dma_start(w1, tw1_d[t, k, :, :])
                    tw1.append(w1)
                if not zero_bias:
                    tb1 = tow.tile([128, KC], F32, tag="tb1")
                    nc.sync.dma_start(tb1, tb1_d[t, :, :])
                tw2 = tow.tile([128, KC], BF16, tag="tw2")
                nc.sync.dma_start(tw2, tw2_d[t, :, :])

                thT = [tow.tile([128, R], BF16, name=f"thT{k}", tag=f"thT{k}", bufs=1)
                       for k in range(KC)]
                for mc in range(KC):
                    for (r0, r1) in RS:
                        ps = psA.tile([128, r1 - r0], F32, name="accT", tag="acc")
                        for k in range(KC):
                            nc.tensor.matmul(
                                ps, tw1[k][:, mc * 128:(mc + 1) * 128], yT[t][:, k * R + r0:k * R + r1],
                                start=(k == 0), stop=(k == KC - 1))
                        if zero_bias:
                            nc.scalar.activation(thT[mc][:, r0:r1], ps, AF.Relu)
                        else:
                            nc.scalar.activation(thT[mc][:, r0:r1], ps, AF.Relu,
                                                 bias=tb1[:, mc:mc + 1])
                for rt in range(NRT):
                    pl = psB.tile([128, 1], F32, name="lg", tag="small", bufs=1)
                    for k in range(KC):
                        nc.tensor.matmul(
                            pl, thT[k][:, rt * 128:(rt + 1) * 128], tw2[:, k:k + 1],
                            start=(k == 0), stop=(k == KC - 1))
                    nc.vector.tensor_copy(logits_sb[:, rt, t:t + 1], pl)

        # ---------------- phase 6: BCE (row-major) ----------------
        lg = logits_sb  # [128, NRT, T]
        if not zero_bias:
            nc.vector.tensor_tensor(lg, lg, tb2_sb.broadcast_to([128, NRT, T]),
                                    OP.add)
        t1_ = perm.tile([128, NRT, T], F32)
        nc.vector.tensor_scalar(t1_, lg, 0.0, None, OP.max)
        t2_ = perm.tile([128, NRT, T], F32)
        nc.vector.tensor_tensor(t2_, lg, labels_rm, OP.mult)
        absl = perm.tile([128, NRT, T], F32)
        nc.scalar.activation(absl, lg, AF.Abs)
        expl = perm.tile([128, NRT, T], F32)
        nc.scalar.activation(expl, absl, AF.Exp, scale=-1.0)
        lp = perm.tile([128, NRT, T], F32)
        nc.scalar.activation(lp, expl, AF.Ln, bias=1.0)
        nc.vector.tensor_tensor(t1_, t1_, t2_, OP.subtract)
        nc.vector.tensor_tensor(t1_, t1_, lp, OP.add)
        bs = perm.tile([128, NRT], F32)
        nc.vector.tensor_reduce(bs, t1_, AX.X, OP.add)
        pb = psB.tile([BSH, 1], F32, name="bsum", tag="small", bufs=1)
        for rt in range(NRT):
            nc.tensor.matmul(pb, sel_sb[rt], bs[:, rt:rt + 1],
                             start=(rt == 0), stop=(rt == NRT - 1))
        tsum = perm.tile([BSH, 1], F32)
        nc.vector.tensor_copy(tsum, pb)

        loss_sb = perm.tile([BSH, 1], F32)
        nc.vector.tensor_scalar(loss_sb, tsum, 1.0 / (T * C), None, OP.mult)
        auxf = perm.tile([BSH, 1], F32)
        nc.vector.tensor_scalar(auxf, auxs, LOSS_COEF, None, OP.mult)
        nc.vector.tensor_tensor(loss_sb, loss_sb, auxf, OP.add)
        nc.sync.dma_start(loss_d[:, :], loss_sb)

    nc.compile()
    return nc


def get_nc_bf16(zero_bias=True):
    key = (zero_bias, tuple(sorted(KNOBS_BF16.items())))
    if key not in _CACHED_BF16:
        _CACHED_BF16[key] = build_nc_bf16(zero_bias)
    return _CACHED_BF16[key]



_SEL_CACHE_BF16 = None


def _sel_mats_bf16():
    """0/1 selector matrices mapping rows r=rt*128+p to samples b=r//30."""
    global _SEL_CACHE_BF16
    if _SEL_CACHE_BF16 is None:
        sel = np.zeros((NRT, 128, BSH), np.float32)
        for rt in range(NRT):
            for p in range(128):
                b = (rt * 128 + p) // C
                sel[rt, p, b] = 1.0
        selt = np.ascontiguousarray(sel.transpose(0, 2, 1))
        _SEL_CACHE_BF16 = (sel, selt)
    return _SEL_CACHE_BF16


def host_prep_bf16(inputs):
    """Shard + cast + rearrange the full inputs into 8 per-core in_maps."""
    x = np.asarray(inputs["candidate_cls_embed"], np.float32)
    scores = np.asarray(inputs["scores"], np.float32)
    fc1_w = np.asarray(inputs["fc1_w"], np.float32)
    fc1_b = np.asarray(inputs["fc1_b"], np.float32)
    fc2_w = np.asarray(inputs["fc2_w"], np.float32)
    fc2_b = np.asarray(inputs["fc2_b"], np.float32)
    w_gate = np.asarray(inputs["w_gate"], np.float32)
    expert_w1 = np.asarray(inputs["expert_w1"], np.float32)
    expert_b1 = np.asarray(inputs["expert_b1"], np.float32)
    expert_w2 = np.asarray(inputs["expert_w2"], np.float32)
    expert_b2 = np.asarray(inputs["expert_b2"], np.float32)
    tower_w1 = np.asarray(inputs["tower_w1"], np.float32)
    tower_b1 = np.asarray(inputs["tower_b1"], np.float32)
    tower_w2 = np.asarray(inputs["tower_w2"], np.float32)
    tower_b2 = np.asarray(inputs["tower_b2"], np.float32)

    zero_bias = not (fc1_b.any() or fc2_b.any() or expert_b1.any()
                     or expert_b2.any() or tower_b1.any() or tower_b2.any())

    shared = {
        "fc1w": fc1_w.astype(BF).reshape(KC, 128, H),
        "fc1b": np.ascontiguousarray(fc1_b.reshape(KC, 128).T),
        "fc2w": fc2_w.astype(BF).reshape(KC, 128, H),
        "fc2b": np.ascontiguousarray(fc2_b.reshape(KC, 128).T),
        "wg": np.ascontiguousarray(w_gate.transpose(1, 0, 2).reshape(H, T * E))
              .astype(BF).reshape(KC, 128, T * E),
        "ew1": expert_w1.astype(BF).reshape(E, KC, 128, EH),
        "eb1": np.ascontiguousarray(
            expert_b1.reshape(E, KC, 128).transpose(0, 2, 1)),
        "ew2": expert_w2.astype(BF).reshape(E, KC, 128, H),
        "eb2": expert_b2.astype(BF).reshape(E, 1, H),
        "tw1": tower_w1.astype(BF).reshape(T, KC, 128, TH),
        "tb1": np.ascontiguousarray(
            tower_b1.reshape(T, KC, 128).transpose(0, 2, 1)),
        "tw2": np.ascontiguousarray(
            tower_w2.reshape(T, KC, 128).transpose(0, 2, 1)).astype(BF),
        "tb2": np.ascontiguousarray(
            np.broadcast_to(tower_b2[None, :], (128, T))),
        "sel": _sel_mats_bf16()[0],
        "selt": _sel_mats_bf16()[1],
    }
    in_maps = []
    for ci in range(NCORES):
        xs = x[ci * BSH:(ci + 1) * BSH].reshape(R, H)
        xT = np.ascontiguousarray(xs.T).astype(BF).reshape(KC, 128, R)
        m = dict(shared)
        m["xT"] = xT
        sc = np.ascontiguousarray(scores[ci * BSH:(ci + 1) * BSH])
        m["scores"] = sc
        srm = sc.transpose(0, 2, 1).reshape(NRT, 128, T).transpose(1, 0, 2)
        m["srm"] = np.ascontiguousarray(srm)
        in_maps.append(m)
    return in_maps, zero_bias




# ---------------------------------------------------------------------------
# dispatch
# ---------------------------------------------------------------------------
from concourse.bass_utils import run_bass_kernel_spmd  # noqa: E402

_CACHED_FP8 = {}


def _zero_bias(inputs):
    return not any(
        np.asarray(inputs[k], np.float32).any()
        for k in ("fc1_b", "fc2_b", "expert_b1", "expert_b2",
                  "tower_b1", "tower_b2"))


def host_prep(inputs):
    if _zero_bias(inputs):
        return host_prep_fp8(inputs), True
    return host_prep_bf16(inputs)


def get_nc(zero_bias=True, knobs=None):
    if zero_bias:
        key = tuple(sorted((knobs or KNOBS_FP8).items()))
        if key not in _CACHED_FP8:
            _CACHED_FP8[key] = build_nc_fp8(knobs)
        return _CACHED_FP8[key]
    return get_nc_bf16(False)


def kernel(**inputs) -> np.ndarray:
    in_maps, zb = host_prep(inputs)
    nc = get_nc(zb)
    res = run_bass_kernel_spmd(nc, in_maps, list(range(NCORES)))
    losses = np.concatenate([res.results[i]["loss"].reshape(-1)
                             for i in range(NCORES)])
    return np.float32(losses.mean(dtype=np.float64))
